# revision 46
# baseline (speedup 1.0000x reference)
"""GQA attention kernel for 8 Trainium2 NeuronCores.

Sharding: tensor-parallel over heads. Core i handles query heads (2i, 2i+1)
and KV head i//2. Out-proj is row-parallel: each core emits a partial
[S, DIM] output; the host sums the 8 partials and adds the output bias.

v3: all heavy streams bf16 (v2), plus a HAM-aware schedule. The PE clock
gate (HAM) throttles to 4/8 pulses when the PE idles >~3.4us, so the
kernel emits one dense PE stream: projection chunk sc, then attention for
query chunk sc-1, then its out-proj slice, round-robin. RoPE/bias (ACT,
DVE) for chunk sc overlap the attention matmuls of chunk sc-1. Input DMAs
load contract-dim pairs ([128, 2, 512] = 2KB per partition line) to halve
issue count; output rows stage in SBUF and fly as one DMA per 128 rows.

On-chip layouts keep head_dim (128) on partitions and sequence on the free
axis, so QK^T needs no transposes, softmax statistics are PE ones-matmuls,
and the attention weights feed the AV matmul directly from the exp output.
"""

import numpy as np

DIM = 2048
Q_HEADS = 16
KV_HEADS = 4
HEAD_DIM = 128
S = 2048
MAX_LEN = 2048
ROPE_THETA = 10000.0
ROPE_FACTOR = 8.0
N_CORES = 8
HEADS_PER_CORE = Q_HEADS // N_CORES  # 2
SCALE = 1.0 / np.sqrt(HEAD_DIM)
NEG = -1.0e30

_F32R_CACHE = {}


def _bf16(x):
    import ml_dtypes

    return np.ascontiguousarray(np.asarray(x, np.float32).astype(ml_dtypes.bfloat16))


def _rope_cos_sin_T():
    d = HEAD_DIM
    seq_eff = max(S, MAX_LEN)
    base_adj = (ROPE_FACTOR * seq_eff / MAX_LEN - (ROPE_FACTOR - 1.0)) ** (d / (d - 2))
    adjusted_base = ROPE_THETA * base_adj
    inv_freq = 1.0 / adjusted_base ** (np.arange(0, d, 2, dtype=np.float32) / d)
    pos = np.arange(S, dtype=np.float32)
    freqs = pos[:, None] * inv_freq[None, :]
    emb = np.concatenate([freqs, freqs], axis=-1)  # [S, d]
    return (
        np.ascontiguousarray(np.cos(emb).T.astype(np.float32)),  # [d, S]
        np.ascontiguousarray(np.sin(emb).T.astype(np.float32)),
    )


def _masks():
    # additive masks for the 4 diagonal 128x512 blocks: block r covers keys
    # [128r, 128r+128) against queries [0, 512) within a 512-query chunk.
    k = np.arange(128)[:, None]
    q = np.arange(512)[None, :]
    m = np.zeros((128, 4, 512), np.float32)
    for r in range(4):
        m[:, r, :] = np.where(128 * r + k > q, NEG, 0.0).astype(np.float32)
    return np.ascontiguousarray(m.reshape(128, 4 * 512))


def _build_program():
    import concourse.bass as bass
    import concourse.tile as tile
    from concourse import mybir
    import bass_rust
    from concourse.vector_clock import ScopedClock
    from concourse.masks import make_identity

    # --- workaround: walrus CTRL instructions accept a single sync wait;
    # split the TileContext end-drain waits across one SP nop each.
    def _patched_drain_and_barrier(self, tick_clock, wait_clock):
        nop0 = self.nc.sync.nop(nofuse=True)
        wait_clock.add_sem_waits(nop0.ins, ScopedClock({None: tick_clock.global_clock}))
        si = nop0.ins.sync_info
        ws = list(si.on_wait) if si is not None else []
        if len(ws) > 1:
            nop0.ins.sync_info = bass_rust.SyncInfo(
                on_wait=ws[:1], on_update=list(si.on_update))
            for i in range(1, len(ws)):
                nop = self.nc.sync.nop(nofuse=True)
                nop.ins.sync_info = bass_rust.SyncInfo(on_wait=ws[i:i + 1], on_update=[])
        self.nc.sync.drain()
        self.nc.all_engine_barrier()
        popped = self.nc._tile_sem_poison_stack.pop()
        assert popped is self._sem_poison
        self.nc.clear_and_free_semaphores(list(self.sems.allocated().values()))
        self.nc.all_engine_barrier()

    tile.TileContext._drain_and_barrier = _patched_drain_and_barrier

    def _split_multi_waits(nc):
        # this walrus build accepts a single sync-wait slot on several
        # instruction encodings; peel extra waits onto same-engine NoOps.
        cnt = 0
        for f in nc.m.functions:
            for bb in f.blocks:
                new_l = []
                for inst in bb.instructions:
                    si = inst.sync_info
                    ws = list(si.on_wait) if si is not None else []
                    if len(ws) > 1:
                        for w in ws[:-1]:
                            nop = mybir.InstNoOp(
                                name=f"{inst.name}_wsplit{cnt}", engine=inst.engine,
                                bass_nofuse=True,
                                sync_info=mybir.SyncInfo(on_wait=[w], on_update=[]))
                            nc.register_instruction(nop, overwrite=True)
                            new_l.append(nop)
                            cnt += 1
                        inst.sync_info = mybir.SyncInfo(
                            on_wait=[ws[-1]], on_update=list(si.on_update))
                    new_l.append(inst)
                bb.instructions = new_l

    f32 = mybir.dt.float32
    bf16 = mybir.dt.bfloat16
    AF = mybir.ActivationFunctionType
    OP = mybir.AluOpType

    nc = bass.Bass()
    # host-pretiled layouts: every DMA reads long contiguous per-partition
    # runs (2KB input lines, 8KB weight lines). The v3 trace showed the
    # single DGE queue capped at ~160GB/s avg on 1KB/512B scattered packets.
    qT_in = nc.dram_tensor("queryT", [8, 4, 128, 2, 512], bf16, kind="ExternalInput")
    kT_in = nc.dram_tensor("keyT", [8, 4, 128, 2, 512], bf16, kind="ExternalInput")
    vT_in = nc.dram_tensor("valueT", [8, 4, 128, 2, 512], bf16, kind="ExternalInput")
    wq_in = nc.dram_tensor("wqT", [128, 16, 256], bf16, kind="ExternalInput")
    wk_in = nc.dram_tensor("wkT", [128, 16, 128], bf16, kind="ExternalInput")
    wv_in = nc.dram_tensor("wvT", [128, 16, 128], bf16, kind="ExternalInput")
    wo_in = nc.dram_tensor("woT", [128, 2, DIM], bf16, kind="ExternalInput")
    bq_in = nc.dram_tensor("bq_col", [128, 2], f32, kind="ExternalInput")
    bk_in = nc.dram_tensor("bk_col", [128, 1], f32, kind="ExternalInput")
    bv_in = nc.dram_tensor("bv_col", [128, 1], f32, kind="ExternalInput")
    cos_in = nc.dram_tensor("cosT", [128, S], f32, kind="ExternalInput")
    sin_in = nc.dram_tensor("sinT", [128, S], f32, kind="ExternalInput")
    mask_in = nc.dram_tensor("masks", [128, 4 * 512], f32, kind="ExternalInput")
    out_dram = nc.dram_tensor("partial", [S, DIM], bf16, kind="ExternalOutput")

    with tile.TileContext(nc) as tc:
        with (
            tc.tile_pool(name="const", bufs=1) as cpool,
            tc.tile_pool(name="stream", bufs=6) as spool,
            tc.tile_pool(name="work", bufs=2) as wpool,
            tc.tile_pool(name="acts", bufs=1) as apool,
            tc.tile_pool(name="attn", bufs=2) as atpool,
            tc.tile_pool(name="ps1", bufs=1, space="PSUM") as ps1,
            tc.tile_pool(name="ps2", bufs=2, space="PSUM") as ps2,
        ):
            # ---- constants / weights. Projection weights lead on sync
            # (first matmuls gate on them + the first input tiles); bulk
            # constants (needed >=25us in) issue from scalar at t=0 while
            # the ACT queue is still empty.
            # only the cc=0,1 weight slivers preload; the rest stream one
            # contract-pair ahead of the inputs inside proj_mm(0), so the
            # first matmul gates on ~400KB instead of 3MB.
            wq_sb = cpool.tile([128, 16, 256], bf16)
            nc.sync.dma_start(wq_sb[:, 0:2], wq_in[:, 0:2])
            wk_sb = cpool.tile([128, 16, 128], bf16)
            nc.sync.dma_start(wk_sb[:, 0:2], wk_in[:, 0:2])
            wv_sb = cpool.tile([128, 16, 128], bf16)
            nc.sync.dma_start(wv_sb[:, 0:2], wv_in[:, 0:2])
            wo_sb = cpool.tile([128, 2, DIM], bf16)
            nc.scalar.dma_start(wo_sb[:], wo_in[:])
            bq_sb = cpool.tile([128, 2], f32)
            nc.scalar.dma_start(bq_sb[:], bq_in[:])
            bk_sb = cpool.tile([128, 1], f32)
            nc.scalar.dma_start(bk_sb[:], bk_in[:])
            bv_sb = cpool.tile([128, 1], f32)
            nc.scalar.dma_start(bv_sb[:], bv_in[:])
            cos_sb = cpool.tile([128, S], f32)
            nc.scalar.dma_start(cos_sb[:], cos_in[:])
            sin_sb = cpool.tile([128, S], f32)
            nc.scalar.dma_start(sin_sb[:], sin_in[:])
            mask_sb = cpool.tile([128, 4, 512], f32)
            nc.scalar.dma_start(mask_sb[:], mask_in.rearrange("p (r q) -> p r q", r=4))
            ones_f = cpool.tile([128, 128], f32)
            nc.vector.memset(ones_f[:], 1.0)
            ones_mat = cpool.tile([128, 128], bf16)
            nc.vector.tensor_copy(out=ones_mat[:], in_=ones_f[:])
            ident = cpool.tile([128, 128], f32)
            make_identity(nc, ident[:])
            # HAM pre-warm: the PE clock gate boots at 4/8 pulses and only
            # releases after ~4us of sustained matmul activity, but the
            # first ~15us is DMA-paced sputter, so the early phases run at
            # half clock. A chain of fp32 dummy matmuls (4 cycles/row) on
            # constant SBUF tiles runs in the DMA shadow, bridges until the
            # first input tiles land, and flips the gate to 8/8 up front.
            warm = cpool.tile([128, 512], f32)
            nc.vector.memset(warm[:], 0.0)
            for _w in range(8):
                pw = ps2.tile([128, 512], f32, tag="sT", name=f"warm{_w}")
                nc.tensor.matmul(pw[:], ident[:], warm[:], start=True, stop=True)

            # ---- persistent per-chunk activations (bf16 matmul operands)
            q_rot = [[apool.tile([128, 512], bf16, tag=f"qrot{h}_{c}", name=f"qrot{h}_{c}")
                      for c in range(4)] for h in range(2)]
            k_rot = [apool.tile([128, 512], bf16, tag=f"krot{c}", name=f"krot{c}")
                     for c in range(4)]
            v_sb = [apool.tile([128, 512], bf16, tag=f"vsb{c}", name=f"vsb{c}")
                    for c in range(4)]
            ctxT = [[apool.tile([128, 512], bf16, tag=f"ctx{h}_{c}", name=f"ctx{h}_{c}")
                     for c in range(4)] for h in range(2)]

            def rope(dst, raw, sc):
                # dst = raw*cos + swap(raw)*sinMod; sinMod has the -1 on the
                # low half baked in host-side (rotate_half sign).
                # math in f32, single rounding into the bf16 dst.
                ssl = slice(sc * 512, sc * 512 + 512)
                swp = wpool.tile([128, 512], f32, tag="ropeswp")
                nc.vector.tensor_copy(out=swp[0:64, :], in_=raw[64:128, :])
                nc.vector.tensor_copy(out=swp[64:128, :], in_=raw[0:64, :])
                tmp = wpool.tile([128, 512], f32, tag="ropetmp")
                nc.vector.tensor_tensor(tmp[:], swp[:], sin_sb[:, ssl], OP.mult)
                acc = wpool.tile([128, 512], f32, tag="ropeacc")
                nc.vector.tensor_tensor(acc[:], raw[:], cos_sb[:, ssl], OP.mult)
                nc.vector.tensor_tensor(acc[:], acc[:], tmp[:], OP.add)
                nc.vector.tensor_copy(out=dst[:], in_=acc[:])

            def proj_mm(sc, first=False, scores=None):
                # projection matmuls for sequence chunk sc; returns live
                # psums. scores=(qc, attnTs, pairs): interleave the previous
                # chunk's score/exp matmuls (SBUF-only, no DMA deps) after
                # each contract-pair so the PE never stalls on input tiles.
                ssl = slice(sc * 512, sc * 512 + 512)
                pq0 = ps1.tile([128, 512], f32, tag="A")
                pq1 = ps1.tile([128, 512], f32, tag="B")
                pk = ps1.tile([128, 512], f32, tag="C")
                pv = ps1.tile([128, 512], f32, tag="D")
                for cp in range(8):
                    if scores is not None:
                        qc_s, attnTs, pairs = scores
                        for h_s, kt_s in pairs[cp * len(pairs) // 8:(cp + 1) * len(pairs) // 8]:
                            emit_score(h_s, kt_s, qc_s, attnTs[h_s])
                    if first and cp < 7:
                        lo, hi = 2 * cp + 2, 2 * cp + 4
                        nc.sync.dma_start(wq_sb[:, lo:hi], wq_in[:, lo:hi])
                        nc.sync.dma_start(wk_sb[:, lo:hi], wk_in[:, lo:hi])
                        nc.sync.dma_start(wv_sb[:, lo:hi], wv_in[:, lo:hi])
                    qt = spool.tile([128, 2, 512], bf16, tag="qs")
                    nc.sync.dma_start(qt[:], qT_in[cp, sc])
                    kt_ = spool.tile([128, 2, 512], bf16, tag="ks")
                    nc.sync.dma_start(kt_[:], kT_in[cp, sc])
                    vt = spool.tile([128, 2, 512], bf16, tag="vs")
                    nc.sync.dma_start(vt[:], vT_in[cp, sc])
                    for t in range(2):
                        cc = 2 * cp + t
                        st, sp = cc == 0, cc == 15
                        nc.tensor.matmul(pq0[:], wq_sb[:, cc, 0:128],
                                         qt[:, t], start=st, stop=sp)
                        nc.tensor.matmul(pq1[:], wq_sb[:, cc, 128:256],
                                         qt[:, t], start=st, stop=sp)
                        nc.tensor.matmul(pk[:], wk_sb[:, cc],
                                         kt_[:, t], start=st, stop=sp)
                        nc.tensor.matmul(pv[:], wv_sb[:, cc],
                                         vt[:, t], start=st, stop=sp)
                return pq0, pq1, pk, pv

            def proj_epilogue(sc, pq0, pq1, pk, pv):
                # All four biases first (frees the proj psum banks early),
                # then the v transposes + their DVE copies at the HEAD of the
                # DVE queue, then RoPE. With ropes first, transpose j>=1
                # stalled the whole PE FIFO ~3us per phase waiting for its
                # psum slot behind ~10us of queued RoPE (the v3 phase gaps).
                q0_raw = wpool.tile([128, 512], f32, tag="rawq0")
                nc.scalar.activation(q0_raw[:], pq0[:], AF.Identity, bias=bq_sb[:, 0:1])
                q1_raw = wpool.tile([128, 512], f32, tag="rawq1")
                nc.scalar.activation(q1_raw[:], pq1[:], AF.Identity, bias=bq_sb[:, 1:2])
                k_raw = wpool.tile([128, 512], f32, tag="rawk")
                nc.scalar.activation(k_raw[:], pk[:], AF.Identity, bias=bk_sb[:])
                v_raw = wpool.tile([128, 512], f32, tag="rawv")
                nc.scalar.activation(v_raw[:], pv[:], AF.Identity, bias=bv_sb[:])
                for j in range(4):
                    ptr = ps1.tile([128, 128], f32, tag="A")
                    nc.tensor.transpose(ptr[:], v_raw[:, j * 128:(j + 1) * 128], ident[:])
                    nc.vector.tensor_copy(
                        out=v_sb[sc][:, j * 128:j * 128 + 128], in_=ptr[:])
                return q0_raw, q1_raw, k_raw

            def proj_ropes(sc, q0_raw, q1_raw, k_raw):
                # emitted LAST in each phase: the ~12us of RoPE DVE work sits
                # behind the attention recip/copies in the DVE FIFO, and
                # finishes during the next proj chunk's DMA-paced matmuls.
                rope(q_rot[0][sc], q0_raw, sc)
                rope(q_rot[1][sc], q1_raw, sc)
                rope(k_rot[sc], k_raw, sc)

            def emit_score(h, kt, qc, attnT):
                pst = ps2.tile([128, 512], f32, tag="sT")
                nc.tensor.matmul(
                    pst[:], k_rot[kt // 4][:, (kt % 4) * 128:(kt % 4) * 128 + 128],
                    q_rot[h][qc][:], start=True, stop=True)
                r = kt - 4 * qc
                if r >= 0:
                    nc.vector.tensor_tensor(pst[:], pst[:], mask_sb[:, r], OP.add)
                nc.scalar.activation(attnT[:, kt], pst[:], AF.Exp, scale=float(SCALE))

            def attn(qc, pre=None):
                # attention for query chunk qc, both heads. If pre is given,
                # the scores/exps were already emitted interleaved into the
                # projection stream; only the sums/normalize remain.
                n_kt = 4 * (qc + 1)
                for h in range(2):
                    if pre is not None:
                        attnT = pre[h]
                    else:
                        attnT = atpool.tile([128, 16, 512], bf16, tag="attnT")
                        for kt in range(n_kt):
                            emit_score(h, kt, qc, attnT)
                    psum = ps1.tile([128, 512], f32, tag="C" if h == 0 else "A")
                    pctx = ps1.tile([128, 512], f32, tag="B" if h == 0 else "D")
                    for kt in range(n_kt):
                        nc.tensor.matmul(psum[:], ones_mat[:],
                                         attnT[:, kt],
                                         start=kt == 0, stop=kt == n_kt - 1)
                        nc.tensor.matmul(pctx[:], v_sb[kt // 4][:, (kt % 4) * 128:(kt % 4) * 128 + 128],
                                         attnT[:, kt],
                                         start=kt == 0, stop=kt == n_kt - 1)
                    # normalize in 128-column groups: out-proj tile st only
                    # needs ctxT columns (st%4)*128..+128, so releasing each
                    # group early unhides the 3.4us full-width reciprocal
                    # from the phase-end critical path.
                    for g4 in range(4):
                        csl = slice(g4 * 128, g4 * 128 + 128)
                        bc_sb = wpool.tile([128, 128], f32, tag="bc")
                        nc.vector.reciprocal(out=bc_sb[:], in_=psum[:, csl])
                        nc.vector.tensor_tensor(
                            ctxT[h][qc][:, csl], pctx[:, csl], bc_sb[:], OP.mult)

            def outproj(qc):
                # out-proj rows for the 4 seq tiles of query chunk qc
                for st in range(4 * qc, 4 * qc + 4):
                    tsl = slice((st % 4) * 128, (st % 4) * 128 + 128)
                    ot = wpool.tile([128, 2048], bf16, tag="ot")
                    for ec in range(4):
                        esl = slice(ec * 512, ec * 512 + 512)
                        po = ps2.tile([128, 512], f32, tag="po")
                        nc.tensor.matmul(po[:], ctxT[0][qc][:, tsl],
                                         wo_sb[:, 0, esl], start=True, stop=False)
                        nc.tensor.matmul(po[:], ctxT[1][qc][:, tsl],
                                         wo_sb[:, 1, esl], start=False, stop=True)
                        if ec % 2 == 0:
                            nc.vector.tensor_copy(out=ot[:, esl], in_=po[:])
                        else:
                            nc.scalar.activation(ot[:, esl], po[:], AF.Copy)
                    nc.sync.dma_start(
                        out_dram[st * 128:st * 128 + 128, :], ot[:])

            # ---- HAM-aware interleave: keep the PE stream dense.
            # proj(0), epi(0), proj(1), epi(1)+attn(0)+out(0), proj(2), ...
            ps = proj_mm(0, first=True)
            raws = proj_epilogue(0, *ps)
            proj_ropes(0, *raws)
            for sc in range(1, 4):
                qc = sc - 1
                at0 = atpool.tile([128, 16, 512], bf16, tag="attnT", name=f"at0_{sc}")
                at1 = atpool.tile([128, 16, 512], bf16, tag="attnT", name=f"at1_{sc}")
                pairs = [(h, kt) for h in range(2) for kt in range(4 * (qc + 1))]
                ps = proj_mm(sc, scores=(qc, [at0, at1], pairs))
                raws = proj_epilogue(sc, *ps)
                attn(qc, pre=[at0, at1])
                outproj(qc)
                proj_ropes(sc, *raws)
            attn(3)
            outproj(3)
    _split_multi_waits(nc)
    return nc


def kernel(query, key, value, Wq, bq, Wk, bk, Wv, bv, Wo, bo):
    from concourse.bass_utils import run_bass_kernel_spmd

    query = np.asarray(query, np.float32)
    key = np.asarray(key, np.float32)
    value = np.asarray(value, np.float32)
    B = query.shape[0]

    def _tile_in(x):
        # [S, DIM] -> [cp, sc, ci, two, s]: per-(cp, sc) DMA block is one
        # contiguous [128, 1024] run (2KB per partition line).
        a = _bf16(x.reshape(S, DIM).T).reshape(8, 2, 128, 4, 512)
        return np.ascontiguousarray(a.transpose(0, 3, 2, 1, 4))

    qT = _tile_in(query)
    kT = _tile_in(key)
    vT = _tile_in(value)
    cosT, sinT = _rope_cos_sin_T()
    sinT = sinT.copy()
    sinT[0:64, :] *= -1.0  # rotate_half: low half gets -x2*sin
    sinT = np.ascontiguousarray(sinT)
    masks = _masks()

    if "nc" not in _F32R_CACHE:
        _F32R_CACHE["nc"] = _build_program()
    nc = _F32R_CACHE["nc"]

    in_maps = []
    for i in range(N_CORES):
        g = i // 2
        # weight slices pretiled to [ci, co, d] / [d, h, e]: contiguous
        # multi-KB per-partition runs for the preload DMAs.
        Wq_s = np.ascontiguousarray(
            _bf16(np.asarray(Wq, np.float32)[256 * i:256 * (i + 1), :].T)
            .reshape(16, 128, 256).transpose(1, 0, 2))
        Wk_s = np.ascontiguousarray(
            _bf16(np.asarray(Wk, np.float32)[128 * g:128 * (g + 1), :].T)
            .reshape(16, 128, 128).transpose(1, 0, 2))
        Wv_s = np.ascontiguousarray(
            _bf16(np.asarray(Wv, np.float32)[128 * g:128 * (g + 1), :].T)
            .reshape(16, 128, 128).transpose(1, 0, 2))
        Wo_s = np.ascontiguousarray(
            _bf16(np.asarray(Wo, np.float32)[:, 256 * i:256 * (i + 1)].T)
            .reshape(2, 128, DIM).transpose(1, 0, 2))
        bq_c = np.ascontiguousarray(np.asarray(bq, np.float32)[256 * i:256 * (i + 1)].reshape(2, 128).T)
        bk_c = np.asarray(bk, np.float32)[128 * g:128 * (g + 1)].reshape(128, 1)
        bv_c = np.asarray(bv, np.float32)[128 * g:128 * (g + 1)].reshape(128, 1)
        in_maps.append({
            "queryT": qT, "keyT": kT, "valueT": vT,
            "wqT": Wq_s, "wkT": Wk_s, "wvT": Wv_s, "woT": Wo_s,
            "bq_col": bq_c, "bk_col": np.ascontiguousarray(bk_c),
            "bv_col": np.ascontiguousarray(bv_c),
            "cosT": cosT, "sinT": sinT, "masks": masks,
        })

    _F32R_CACHE["in_maps"] = in_maps
    globals()["_LAST_IN_MAPS"] = in_maps
    res = run_bass_kernel_spmd(nc, in_maps, list(range(N_CORES)))
    out = res.results[0]["partial"].astype(np.float32)
    for i in range(1, N_CORES):
        out = out + res.results[i]["partial"].astype(np.float32)
    out = out + np.asarray(bo, np.float32)[None, :]
    return out.reshape(B, S, DIM).astype(np.float32)


# revision 48
# speedup vs baseline: 1.0216x; 1.0216x over previous
"""GQA attention kernel for 8 Trainium2 NeuronCores.

Sharding: tensor-parallel over heads. Core i handles query heads (2i, 2i+1)
and KV head i//2. Out-proj is row-parallel: each core emits a partial
[S, DIM] output; the host sums the 8 partials and adds the output bias.

v3: all heavy streams bf16 (v2), plus a HAM-aware schedule. The PE clock
gate (HAM) throttles to 4/8 pulses when the PE idles >~3.4us, so the
kernel emits one dense PE stream: projection chunk sc, then attention for
query chunk sc-1, then its out-proj slice, round-robin. RoPE/bias (ACT,
DVE) for chunk sc overlap the attention matmuls of chunk sc-1. Input DMAs
load contract-dim pairs ([128, 2, 512] = 2KB per partition line) to halve
issue count; output rows stage in SBUF and fly as one DMA per 128 rows.

On-chip layouts keep head_dim (128) on partitions and sequence on the free
axis, so QK^T needs no transposes, softmax statistics are PE ones-matmuls,
and the attention weights feed the AV matmul directly from the exp output.
"""

import numpy as np

DIM = 2048
Q_HEADS = 16
KV_HEADS = 4
HEAD_DIM = 128
S = 2048
MAX_LEN = 2048
ROPE_THETA = 10000.0
ROPE_FACTOR = 8.0
N_CORES = 8
HEADS_PER_CORE = Q_HEADS // N_CORES  # 2
SCALE = 1.0 / np.sqrt(HEAD_DIM)
NEG = -1.0e30

_F32R_CACHE = {}


def _bf16(x):
    import ml_dtypes

    return np.ascontiguousarray(np.asarray(x, np.float32).astype(ml_dtypes.bfloat16))


def _rope_cos_sin_T():
    d = HEAD_DIM
    seq_eff = max(S, MAX_LEN)
    base_adj = (ROPE_FACTOR * seq_eff / MAX_LEN - (ROPE_FACTOR - 1.0)) ** (d / (d - 2))
    adjusted_base = ROPE_THETA * base_adj
    inv_freq = 1.0 / adjusted_base ** (np.arange(0, d, 2, dtype=np.float32) / d)
    pos = np.arange(S, dtype=np.float32)
    freqs = pos[:, None] * inv_freq[None, :]
    emb = np.concatenate([freqs, freqs], axis=-1)  # [S, d]
    return (
        np.ascontiguousarray(np.cos(emb).T.astype(np.float32)),  # [d, S]
        np.ascontiguousarray(np.sin(emb).T.astype(np.float32)),
    )


def _masks():
    # additive masks for the 4 diagonal 128x512 blocks: block r covers keys
    # [128r, 128r+128) against queries [0, 512) within a 512-query chunk.
    k = np.arange(128)[:, None]
    q = np.arange(512)[None, :]
    m = np.zeros((128, 4, 512), np.float32)
    for r in range(4):
        m[:, r, :] = np.where(128 * r + k > q, NEG, 0.0).astype(np.float32)
    return np.ascontiguousarray(m.reshape(128, 4 * 512))


def _build_program():
    import concourse.bass as bass
    import concourse.tile as tile
    from concourse import mybir
    import bass_rust
    from concourse.vector_clock import ScopedClock
    from concourse.masks import make_identity

    # --- workaround: walrus CTRL instructions accept a single sync wait;
    # split the TileContext end-drain waits across one SP nop each.
    def _patched_drain_and_barrier(self, tick_clock, wait_clock):
        nop0 = self.nc.sync.nop(nofuse=True)
        wait_clock.add_sem_waits(nop0.ins, ScopedClock({None: tick_clock.global_clock}))
        si = nop0.ins.sync_info
        ws = list(si.on_wait) if si is not None else []
        if len(ws) > 1:
            nop0.ins.sync_info = bass_rust.SyncInfo(
                on_wait=ws[:1], on_update=list(si.on_update))
            for i in range(1, len(ws)):
                nop = self.nc.sync.nop(nofuse=True)
                nop.ins.sync_info = bass_rust.SyncInfo(on_wait=ws[i:i + 1], on_update=[])
        self.nc.sync.drain()
        self.nc.all_engine_barrier()
        popped = self.nc._tile_sem_poison_stack.pop()
        assert popped is self._sem_poison
        self.nc.clear_and_free_semaphores(list(self.sems.allocated().values()))
        self.nc.all_engine_barrier()

    tile.TileContext._drain_and_barrier = _patched_drain_and_barrier

    def _split_multi_waits(nc):
        # this walrus build accepts a single sync-wait slot on several
        # instruction encodings; peel extra waits onto same-engine NoOps.
        cnt = 0
        for f in nc.m.functions:
            for bb in f.blocks:
                new_l = []
                for inst in bb.instructions:
                    si = inst.sync_info
                    ws = list(si.on_wait) if si is not None else []
                    if len(ws) > 1:
                        for w in ws[:-1]:
                            nop = mybir.InstNoOp(
                                name=f"{inst.name}_wsplit{cnt}", engine=inst.engine,
                                bass_nofuse=True,
                                sync_info=mybir.SyncInfo(on_wait=[w], on_update=[]))
                            nc.register_instruction(nop, overwrite=True)
                            new_l.append(nop)
                            cnt += 1
                        inst.sync_info = mybir.SyncInfo(
                            on_wait=[ws[-1]], on_update=list(si.on_update))
                    new_l.append(inst)
                bb.instructions = new_l

    f32 = mybir.dt.float32
    bf16 = mybir.dt.bfloat16
    AF = mybir.ActivationFunctionType
    OP = mybir.AluOpType

    nc = bass.Bass()
    # host-pretiled layouts: every DMA reads long contiguous per-partition
    # runs (2KB input lines, 8KB weight lines). The v3 trace showed the
    # single DGE queue capped at ~160GB/s avg on 1KB/512B scattered packets.
    qT_in = nc.dram_tensor("queryT", [8, 4, 128, 2, 512], bf16, kind="ExternalInput")
    kT_in = nc.dram_tensor("keyT", [8, 4, 128, 2, 512], bf16, kind="ExternalInput")
    vT_in = nc.dram_tensor("valueT", [8, 4, 128, 2, 512], bf16, kind="ExternalInput")
    wq_in = nc.dram_tensor("wqT", [128, 16, 256], bf16, kind="ExternalInput")
    wk_in = nc.dram_tensor("wkT", [128, 16, 128], bf16, kind="ExternalInput")
    wv_in = nc.dram_tensor("wvT", [128, 16, 128], bf16, kind="ExternalInput")
    wo_in = nc.dram_tensor("woT", [128, 2, DIM], bf16, kind="ExternalInput")
    bq_in = nc.dram_tensor("bq_col", [128, 2], f32, kind="ExternalInput")
    bk_in = nc.dram_tensor("bk_col", [128, 1], f32, kind="ExternalInput")
    bv_in = nc.dram_tensor("bv_col", [128, 1], f32, kind="ExternalInput")
    cos_in = nc.dram_tensor("cosT", [128, S], f32, kind="ExternalInput")
    sin_in = nc.dram_tensor("sinT", [128, S], f32, kind="ExternalInput")
    mask_in = nc.dram_tensor("masks", [128, 4 * 512], f32, kind="ExternalInput")
    out_dram = nc.dram_tensor("partial", [S, DIM], bf16, kind="ExternalOutput")

    with tile.TileContext(nc) as tc:
        with (
            tc.tile_pool(name="const", bufs=1) as cpool,
            tc.tile_pool(name="stream", bufs=6) as spool,
            tc.tile_pool(name="work", bufs=2) as wpool,
            tc.tile_pool(name="acts", bufs=1) as apool,
            tc.tile_pool(name="attn", bufs=2) as atpool,
            tc.tile_pool(name="ps1", bufs=1, space="PSUM") as ps1,
            tc.tile_pool(name="ps2", bufs=2, space="PSUM") as ps2,
        ):
            # ---- constants / weights. Projection weights lead on sync
            # (first matmuls gate on them + the first input tiles); bulk
            # constants (needed >=25us in) issue from scalar at t=0 while
            # the ACT queue is still empty.
            # only the cc=0,1 weight slivers preload; the rest stream one
            # contract-pair ahead of the inputs inside proj_mm(0), so the
            # first matmul gates on ~400KB instead of 3MB.
            wq_sb = cpool.tile([128, 16, 256], bf16)
            nc.sync.dma_start(wq_sb[:, 0:2], wq_in[:, 0:2])
            wk_sb = cpool.tile([128, 16, 128], bf16)
            nc.sync.dma_start(wk_sb[:, 0:2], wk_in[:, 0:2])
            wv_sb = cpool.tile([128, 16, 128], bf16)
            nc.sync.dma_start(wv_sb[:, 0:2], wv_in[:, 0:2])
            wo_sb = cpool.tile([128, 2, DIM], bf16)
            nc.scalar.dma_start(wo_sb[:], wo_in[:])
            bq_sb = cpool.tile([128, 2], f32)
            nc.scalar.dma_start(bq_sb[:], bq_in[:])
            bk_sb = cpool.tile([128, 1], f32)
            nc.scalar.dma_start(bk_sb[:], bk_in[:])
            bv_sb = cpool.tile([128, 1], f32)
            nc.scalar.dma_start(bv_sb[:], bv_in[:])
            cos_sb = cpool.tile([128, S], f32)
            nc.scalar.dma_start(cos_sb[:], cos_in[:])
            sin_sb = cpool.tile([128, S], f32)
            nc.scalar.dma_start(sin_sb[:], sin_in[:])
            mask_sb = cpool.tile([128, 4, 512], f32)
            nc.scalar.dma_start(mask_sb[:], mask_in.rearrange("p (r q) -> p r q", r=4))
            # HAM pre-warm: the PE clock gate boots at 4/8 and releases only
            # after ~4us of sustained matmul activity; the first ~30us is
            # DMA-paced sputter at half clock. A zero-matmul chain starting
            # ~2.5us (gated only on two DVE memsets) flips the gate before
            # the real stream begins; keep-warm fillers inside the first two
            # projection phases stop the >3.4us idle windows from
            # re-throttling it (v19's trace confirmed the flip at t=14us but
            # lost it to a 4.7us input-wait gap).
            wstat = cpool.tile([128, 128], f32)
            nc.vector.memset(wstat[:], 0.0)
            warm = cpool.tile([128, 512], f32)
            nc.vector.memset(warm[:], 0.0)
            for _w in range(10):
                pw = ps2.tile([128, 512], f32, tag="po", name=f"warm{_w}")
                nc.tensor.matmul(pw[:], wstat[:], warm[:], start=True, stop=True)
            ones_f = cpool.tile([128, 128], f32)
            nc.vector.memset(ones_f[:], 1.0)
            ones_mat = cpool.tile([128, 128], bf16)
            nc.vector.tensor_copy(out=ones_mat[:], in_=ones_f[:])
            ident = cpool.tile([128, 128], f32)
            make_identity(nc, ident[:])

            # ---- persistent per-chunk activations (bf16 matmul operands)
            q_rot = [[apool.tile([128, 512], bf16, tag=f"qrot{h}_{c}", name=f"qrot{h}_{c}")
                      for c in range(4)] for h in range(2)]
            k_rot = [apool.tile([128, 512], bf16, tag=f"krot{c}", name=f"krot{c}")
                     for c in range(4)]
            v_sb = [apool.tile([128, 512], bf16, tag=f"vsb{c}", name=f"vsb{c}")
                    for c in range(4)]
            ctxT = [[apool.tile([128, 512], bf16, tag=f"ctx{h}_{c}", name=f"ctx{h}_{c}")
                     for c in range(4)] for h in range(2)]

            def rope(dst, raw, sc):
                # dst = raw*cos + swap(raw)*sinMod; sinMod has the -1 on the
                # low half baked in host-side (rotate_half sign).
                # math in f32, single rounding into the bf16 dst.
                ssl = slice(sc * 512, sc * 512 + 512)
                swp = wpool.tile([128, 512], f32, tag="ropeswp")
                nc.vector.tensor_copy(out=swp[0:64, :], in_=raw[64:128, :])
                nc.vector.tensor_copy(out=swp[64:128, :], in_=raw[0:64, :])
                tmp = wpool.tile([128, 512], f32, tag="ropetmp")
                nc.vector.tensor_tensor(tmp[:], swp[:], sin_sb[:, ssl], OP.mult)
                acc = wpool.tile([128, 512], f32, tag="ropeacc")
                nc.vector.tensor_tensor(acc[:], raw[:], cos_sb[:, ssl], OP.mult)
                nc.vector.tensor_tensor(acc[:], acc[:], tmp[:], OP.add)
                nc.vector.tensor_copy(out=dst[:], in_=acc[:])

            def proj_mm(sc, first=False, scores=None, fill=False):
                # projection matmuls for sequence chunk sc; returns live
                # psums. scores=(qc, attnTs, pairs): interleave the previous
                # chunk's score/exp matmuls (SBUF-only, no DMA deps) after
                # each contract-pair so the PE never stalls on input tiles.
                ssl = slice(sc * 512, sc * 512 + 512)
                pq0 = ps1.tile([128, 512], f32, tag="A")
                pq1 = ps1.tile([128, 512], f32, tag="B")
                pk = ps1.tile([128, 512], f32, tag="C")
                pv = ps1.tile([128, 512], f32, tag="D")
                for cp in range(8):
                    if scores is not None:
                        qc_s, attnTs, pairs = scores
                        for h_s, kt_s in pairs[cp * len(pairs) // 8:(cp + 1) * len(pairs) // 8]:
                            emit_score(h_s, kt_s, qc_s, attnTs[h_s])
                    if first and cp < 7:
                        lo, hi = 2 * cp + 2, 2 * cp + 4
                        nc.sync.dma_start(wq_sb[:, lo:hi], wq_in[:, lo:hi])
                        nc.sync.dma_start(wk_sb[:, lo:hi], wk_in[:, lo:hi])
                        nc.sync.dma_start(wv_sb[:, lo:hi], wv_in[:, lo:hi])
                    qt = spool.tile([128, 2, 512], bf16, tag="qs")
                    nc.sync.dma_start(qt[:], qT_in[cp, sc])
                    kt_ = spool.tile([128, 2, 512], bf16, tag="ks")
                    nc.sync.dma_start(kt_[:], kT_in[cp, sc])
                    vt = spool.tile([128, 2, 512], bf16, tag="vs")
                    nc.sync.dma_start(vt[:], vT_in[cp, sc])
                    for t in range(2):
                        cc = 2 * cp + t
                        st, sp = cc == 0, cc == 15
                        nc.tensor.matmul(pq0[:], wq_sb[:, cc, 0:128],
                                         qt[:, t], start=st, stop=sp)
                        nc.tensor.matmul(pq1[:], wq_sb[:, cc, 128:256],
                                         qt[:, t], start=st, stop=sp)
                        nc.tensor.matmul(pk[:], wk_sb[:, cc],
                                         kt_[:, t], start=st, stop=sp)
                        nc.tensor.matmul(pv[:], wv_sb[:, cc],
                                         vt[:, t], start=st, stop=sp)
                    if fill and cp >= 1:
                        pw = ps2.tile([128, 512], f32, tag="po",
                                      name=f"fill{sc}_{cp}")
                        nc.tensor.matmul(pw[:], wstat[:], warm[:],
                                         start=True, stop=True)
                return pq0, pq1, pk, pv

            def proj_epilogue(sc, pq0, pq1, pk, pv):
                # All four biases first (frees the proj psum banks early),
                # then the v transposes + their DVE copies at the HEAD of the
                # DVE queue, then RoPE. With ropes first, transpose j>=1
                # stalled the whole PE FIFO ~3us per phase waiting for its
                # psum slot behind ~10us of queued RoPE (the v3 phase gaps).
                q0_raw = wpool.tile([128, 512], f32, tag="rawq0")
                nc.scalar.activation(q0_raw[:], pq0[:], AF.Identity, bias=bq_sb[:, 0:1])
                q1_raw = wpool.tile([128, 512], f32, tag="rawq1")
                nc.scalar.activation(q1_raw[:], pq1[:], AF.Identity, bias=bq_sb[:, 1:2])
                k_raw = wpool.tile([128, 512], f32, tag="rawk")
                nc.scalar.activation(k_raw[:], pk[:], AF.Identity, bias=bk_sb[:])
                v_raw = wpool.tile([128, 512], f32, tag="rawv")
                nc.scalar.activation(v_raw[:], pv[:], AF.Identity, bias=bv_sb[:])
                for j in range(4):
                    ptr = ps1.tile([128, 128], f32, tag="A")
                    nc.tensor.transpose(ptr[:], v_raw[:, j * 128:(j + 1) * 128], ident[:])
                    nc.vector.tensor_copy(
                        out=v_sb[sc][:, j * 128:j * 128 + 128], in_=ptr[:])
                return q0_raw, q1_raw, k_raw

            def proj_ropes(sc, q0_raw, q1_raw, k_raw):
                # emitted LAST in each phase: the ~12us of RoPE DVE work sits
                # behind the attention recip/copies in the DVE FIFO, and
                # finishes during the next proj chunk's DMA-paced matmuls.
                rope(q_rot[0][sc], q0_raw, sc)
                rope(q_rot[1][sc], q1_raw, sc)
                rope(k_rot[sc], k_raw, sc)

            def emit_score(h, kt, qc, attnT):
                pst = ps2.tile([128, 512], f32, tag="sT")
                nc.tensor.matmul(
                    pst[:], k_rot[kt // 4][:, (kt % 4) * 128:(kt % 4) * 128 + 128],
                    q_rot[h][qc][:], start=True, stop=True)
                r = kt - 4 * qc
                if r >= 0:
                    nc.vector.tensor_tensor(pst[:], pst[:], mask_sb[:, r], OP.add)
                nc.scalar.activation(attnT[:, kt], pst[:], AF.Exp, scale=float(SCALE))

            def attn(qc, pre=None):
                # attention for query chunk qc, both heads. If pre is given,
                # the scores/exps were already emitted interleaved into the
                # projection stream; only the sums/normalize remain.
                n_kt = 4 * (qc + 1)
                for h in range(2):
                    if pre is not None:
                        attnT = pre[h]
                    else:
                        attnT = atpool.tile([128, 16, 512], bf16, tag="attnT")
                        for kt in range(n_kt):
                            emit_score(h, kt, qc, attnT)
                    psum = ps1.tile([128, 512], f32, tag="C" if h == 0 else "A")
                    pctx = ps1.tile([128, 512], f32, tag="B" if h == 0 else "D")
                    for kt in range(n_kt):
                        nc.tensor.matmul(psum[:], ones_mat[:],
                                         attnT[:, kt],
                                         start=kt == 0, stop=kt == n_kt - 1)
                        nc.tensor.matmul(pctx[:], v_sb[kt // 4][:, (kt % 4) * 128:(kt % 4) * 128 + 128],
                                         attnT[:, kt],
                                         start=kt == 0, stop=kt == n_kt - 1)
                    # normalize in 128-column groups: out-proj tile st only
                    # needs ctxT columns (st%4)*128..+128, so releasing each
                    # group early unhides the 3.4us full-width reciprocal
                    # from the phase-end critical path.
                    for g4 in range(4):
                        csl = slice(g4 * 128, g4 * 128 + 128)
                        bc_sb = wpool.tile([128, 128], f32, tag="bc")
                        nc.vector.reciprocal(out=bc_sb[:], in_=psum[:, csl])
                        nc.vector.tensor_tensor(
                            ctxT[h][qc][:, csl], pctx[:, csl], bc_sb[:], OP.mult)

            def outproj(qc):
                # out-proj rows for the 4 seq tiles of query chunk qc
                for st in range(4 * qc, 4 * qc + 4):
                    tsl = slice((st % 4) * 128, (st % 4) * 128 + 128)
                    ot = wpool.tile([128, 2048], bf16, tag="ot")
                    for ec in range(4):
                        esl = slice(ec * 512, ec * 512 + 512)
                        po = ps2.tile([128, 512], f32, tag="po")
                        nc.tensor.matmul(po[:], ctxT[0][qc][:, tsl],
                                         wo_sb[:, 0, esl], start=True, stop=False)
                        nc.tensor.matmul(po[:], ctxT[1][qc][:, tsl],
                                         wo_sb[:, 1, esl], start=False, stop=True)
                        if ec % 2 == 0:
                            nc.vector.tensor_copy(out=ot[:, esl], in_=po[:])
                        else:
                            nc.scalar.activation(ot[:, esl], po[:], AF.Copy)
                    nc.sync.dma_start(
                        out_dram[st * 128:st * 128 + 128, :], ot[:])

            # ---- HAM-aware interleave: keep the PE stream dense.
            # proj(0), epi(0), proj(1), epi(1)+attn(0)+out(0), proj(2), ...
            ps = proj_mm(0, first=True, fill=True)
            raws = proj_epilogue(0, *ps)
            proj_ropes(0, *raws)
            for sc in range(1, 4):
                qc = sc - 1
                at0 = atpool.tile([128, 16, 512], bf16, tag="attnT", name=f"at0_{sc}")
                at1 = atpool.tile([128, 16, 512], bf16, tag="attnT", name=f"at1_{sc}")
                pairs = [(h, kt) for h in range(2) for kt in range(4 * (qc + 1))]
                ps = proj_mm(sc, scores=(qc, [at0, at1], pairs), fill=sc == 1)
                raws = proj_epilogue(sc, *ps)
                attn(qc, pre=[at0, at1])
                outproj(qc)
                proj_ropes(sc, *raws)
            attn(3)
            outproj(3)
    _split_multi_waits(nc)
    return nc


def kernel(query, key, value, Wq, bq, Wk, bk, Wv, bv, Wo, bo):
    from concourse.bass_utils import run_bass_kernel_spmd

    query = np.asarray(query, np.float32)
    key = np.asarray(key, np.float32)
    value = np.asarray(value, np.float32)
    B = query.shape[0]

    def _tile_in(x):
        # [S, DIM] -> [cp, sc, ci, two, s]: per-(cp, sc) DMA block is one
        # contiguous [128, 1024] run (2KB per partition line).
        a = _bf16(x.reshape(S, DIM).T).reshape(8, 2, 128, 4, 512)
        return np.ascontiguousarray(a.transpose(0, 3, 2, 1, 4))

    qT = _tile_in(query)
    kT = _tile_in(key)
    vT = _tile_in(value)
    cosT, sinT = _rope_cos_sin_T()
    sinT = sinT.copy()
    sinT[0:64, :] *= -1.0  # rotate_half: low half gets -x2*sin
    sinT = np.ascontiguousarray(sinT)
    masks = _masks()

    if "nc" not in _F32R_CACHE:
        _F32R_CACHE["nc"] = _build_program()
    nc = _F32R_CACHE["nc"]

    in_maps = []
    for i in range(N_CORES):
        g = i // 2
        # weight slices pretiled to [ci, co, d] / [d, h, e]: contiguous
        # multi-KB per-partition runs for the preload DMAs.
        Wq_s = np.ascontiguousarray(
            _bf16(np.asarray(Wq, np.float32)[256 * i:256 * (i + 1), :].T)
            .reshape(16, 128, 256).transpose(1, 0, 2))
        Wk_s = np.ascontiguousarray(
            _bf16(np.asarray(Wk, np.float32)[128 * g:128 * (g + 1), :].T)
            .reshape(16, 128, 128).transpose(1, 0, 2))
        Wv_s = np.ascontiguousarray(
            _bf16(np.asarray(Wv, np.float32)[128 * g:128 * (g + 1), :].T)
            .reshape(16, 128, 128).transpose(1, 0, 2))
        Wo_s = np.ascontiguousarray(
            _bf16(np.asarray(Wo, np.float32)[:, 256 * i:256 * (i + 1)].T)
            .reshape(2, 128, DIM).transpose(1, 0, 2))
        bq_c = np.ascontiguousarray(np.asarray(bq, np.float32)[256 * i:256 * (i + 1)].reshape(2, 128).T)
        bk_c = np.asarray(bk, np.float32)[128 * g:128 * (g + 1)].reshape(128, 1)
        bv_c = np.asarray(bv, np.float32)[128 * g:128 * (g + 1)].reshape(128, 1)
        in_maps.append({
            "queryT": qT, "keyT": kT, "valueT": vT,
            "wqT": Wq_s, "wkT": Wk_s, "wvT": Wv_s, "woT": Wo_s,
            "bq_col": bq_c, "bk_col": np.ascontiguousarray(bk_c),
            "bv_col": np.ascontiguousarray(bv_c),
            "cosT": cosT, "sinT": sinT, "masks": masks,
        })

    _F32R_CACHE["in_maps"] = in_maps
    globals()["_LAST_IN_MAPS"] = in_maps
    res = run_bass_kernel_spmd(nc, in_maps, list(range(N_CORES)))
    out = res.results[0]["partial"].astype(np.float32)
    for i in range(1, N_CORES):
        out = out + res.results[i]["partial"].astype(np.float32)
    out = out + np.asarray(bo, np.float32)[None, :]
    return out.reshape(B, S, DIM).astype(np.float32)


# revision 52
# speedup vs baseline: 1.0267x; 1.0050x over previous
"""GQA attention kernel for 8 Trainium2 NeuronCores.

Sharding: tensor-parallel over heads. Core i handles query heads (2i, 2i+1)
and KV head i//2. Out-proj is row-parallel: each core emits a partial
[S, DIM] output; the host sums the 8 partials and adds the output bias.

v3: all heavy streams bf16 (v2), plus a HAM-aware schedule. The PE clock
gate (HAM) throttles to 4/8 pulses when the PE idles >~3.4us, so the
kernel emits one dense PE stream: projection chunk sc, then attention for
query chunk sc-1, then its out-proj slice, round-robin. RoPE/bias (ACT,
DVE) for chunk sc overlap the attention matmuls of chunk sc-1. Input DMAs
load contract-dim pairs ([128, 2, 512] = 2KB per partition line) to halve
issue count; output rows stage in SBUF and fly as one DMA per 128 rows.

On-chip layouts keep head_dim (128) on partitions and sequence on the free
axis, so QK^T needs no transposes, softmax statistics are PE ones-matmuls,
and the attention weights feed the AV matmul directly from the exp output.
"""

import numpy as np

DIM = 2048
Q_HEADS = 16
KV_HEADS = 4
HEAD_DIM = 128
S = 2048
MAX_LEN = 2048
ROPE_THETA = 10000.0
ROPE_FACTOR = 8.0
N_CORES = 8
HEADS_PER_CORE = Q_HEADS // N_CORES  # 2
SCALE = 1.0 / np.sqrt(HEAD_DIM)
NEG = -1.0e30

_F32R_CACHE = {}


def _bf16(x):
    import ml_dtypes

    return np.ascontiguousarray(np.asarray(x, np.float32).astype(ml_dtypes.bfloat16))


def _rope_cos_sin_T():
    d = HEAD_DIM
    seq_eff = max(S, MAX_LEN)
    base_adj = (ROPE_FACTOR * seq_eff / MAX_LEN - (ROPE_FACTOR - 1.0)) ** (d / (d - 2))
    adjusted_base = ROPE_THETA * base_adj
    inv_freq = 1.0 / adjusted_base ** (np.arange(0, d, 2, dtype=np.float32) / d)
    pos = np.arange(S, dtype=np.float32)
    freqs = pos[:, None] * inv_freq[None, :]
    emb = np.concatenate([freqs, freqs], axis=-1)  # [S, d]
    return (
        np.ascontiguousarray(np.cos(emb).T.astype(np.float32)),  # [d, S]
        np.ascontiguousarray(np.sin(emb).T.astype(np.float32)),
    )


def _masks():
    # additive masks for the 4 diagonal 128x512 blocks: block r covers keys
    # [128r, 128r+128) against queries [0, 512) within a 512-query chunk.
    k = np.arange(128)[:, None]
    q = np.arange(512)[None, :]
    m = np.zeros((128, 4, 512), np.float32)
    for r in range(4):
        m[:, r, :] = np.where(128 * r + k > q, NEG, 0.0).astype(np.float32)
    return np.ascontiguousarray(m.reshape(128, 4 * 512))


def _build_program():
    import concourse.bass as bass
    import concourse.tile as tile
    from concourse import mybir
    import bass_rust
    from concourse.vector_clock import ScopedClock
    from concourse.masks import make_identity

    # --- workaround: walrus CTRL instructions accept a single sync wait;
    # split the TileContext end-drain waits across one SP nop each.
    def _patched_drain_and_barrier(self, tick_clock, wait_clock):
        nop0 = self.nc.sync.nop(nofuse=True)
        wait_clock.add_sem_waits(nop0.ins, ScopedClock({None: tick_clock.global_clock}))
        si = nop0.ins.sync_info
        ws = list(si.on_wait) if si is not None else []
        if len(ws) > 1:
            nop0.ins.sync_info = bass_rust.SyncInfo(
                on_wait=ws[:1], on_update=list(si.on_update))
            for i in range(1, len(ws)):
                nop = self.nc.sync.nop(nofuse=True)
                nop.ins.sync_info = bass_rust.SyncInfo(on_wait=ws[i:i + 1], on_update=[])
        self.nc.sync.drain()
        self.nc.all_engine_barrier()
        popped = self.nc._tile_sem_poison_stack.pop()
        assert popped is self._sem_poison
        self.nc.clear_and_free_semaphores(list(self.sems.allocated().values()))
        self.nc.all_engine_barrier()

    tile.TileContext._drain_and_barrier = _patched_drain_and_barrier

    def _split_multi_waits(nc):
        # this walrus build accepts a single sync-wait slot on several
        # instruction encodings; peel extra waits onto same-engine NoOps.
        cnt = 0
        for f in nc.m.functions:
            for bb in f.blocks:
                new_l = []
                for inst in bb.instructions:
                    si = inst.sync_info
                    ws = list(si.on_wait) if si is not None else []
                    if len(ws) > 1:
                        for w in ws[:-1]:
                            nop = mybir.InstNoOp(
                                name=f"{inst.name}_wsplit{cnt}", engine=inst.engine,
                                bass_nofuse=True,
                                sync_info=mybir.SyncInfo(on_wait=[w], on_update=[]))
                            nc.register_instruction(nop, overwrite=True)
                            new_l.append(nop)
                            cnt += 1
                        inst.sync_info = mybir.SyncInfo(
                            on_wait=[ws[-1]], on_update=list(si.on_update))
                    new_l.append(inst)
                bb.instructions = new_l

    f32 = mybir.dt.float32
    bf16 = mybir.dt.bfloat16
    AF = mybir.ActivationFunctionType
    OP = mybir.AluOpType

    nc = bass.Bass()
    # host-pretiled layouts: every DMA reads long contiguous per-partition
    # runs (2KB input lines, 8KB weight lines). The v3 trace showed the
    # single DGE queue capped at ~160GB/s avg on 1KB/512B scattered packets.
    qT_in = nc.dram_tensor("queryT", [8, 4, 128, 2, 512], bf16, kind="ExternalInput")
    kT_in = nc.dram_tensor("keyT", [8, 4, 128, 2, 512], bf16, kind="ExternalInput")
    vT_in = nc.dram_tensor("valueT", [8, 4, 128, 2, 512], bf16, kind="ExternalInput")
    wq_in = nc.dram_tensor("wqT", [128, 16, 256], bf16, kind="ExternalInput")
    wk_in = nc.dram_tensor("wkT", [128, 16, 128], bf16, kind="ExternalInput")
    wv_in = nc.dram_tensor("wvT", [128, 16, 128], bf16, kind="ExternalInput")
    wo_in = nc.dram_tensor("woT", [128, 2, DIM], bf16, kind="ExternalInput")
    bq_in = nc.dram_tensor("bq_col", [128, 2], f32, kind="ExternalInput")
    bk_in = nc.dram_tensor("bk_col", [128, 1], f32, kind="ExternalInput")
    bv_in = nc.dram_tensor("bv_col", [128, 1], f32, kind="ExternalInput")
    cos_in = nc.dram_tensor("cosT", [128, S], f32, kind="ExternalInput")
    sin_in = nc.dram_tensor("sinT", [128, S], f32, kind="ExternalInput")
    mask_in = nc.dram_tensor("masks", [128, 4 * 512], f32, kind="ExternalInput")
    out_dram = nc.dram_tensor("partial", [S, DIM], bf16, kind="ExternalOutput")

    with tile.TileContext(nc) as tc:
        with (
            tc.tile_pool(name="const", bufs=1) as cpool,
            tc.tile_pool(name="stream", bufs=6) as spool,
            tc.tile_pool(name="work", bufs=2) as wpool,
            tc.tile_pool(name="acts", bufs=1) as apool,
            tc.tile_pool(name="attn", bufs=2) as atpool,
            tc.tile_pool(name="ps1", bufs=1, space="PSUM") as ps1,
            tc.tile_pool(name="ps2", bufs=2, space="PSUM") as ps2,
        ):
            # ---- constants / weights. Projection weights lead on sync
            # (first matmuls gate on them + the first input tiles); bulk
            # constants (needed >=25us in) issue from scalar at t=0 while
            # the ACT queue is still empty.
            # only the cc=0,1 weight slivers preload; the rest stream one
            # contract-pair ahead of the inputs inside proj_mm(0), so the
            # first matmul gates on ~400KB instead of 3MB.
            wq_sb = cpool.tile([128, 16, 256], bf16)
            nc.sync.dma_start(wq_sb[:, 0:2], wq_in[:, 0:2])
            wk_sb = cpool.tile([128, 16, 128], bf16)
            nc.sync.dma_start(wk_sb[:, 0:2], wk_in[:, 0:2])
            wv_sb = cpool.tile([128, 16, 128], bf16)
            nc.sync.dma_start(wv_sb[:, 0:2], wv_in[:, 0:2])
            wo_sb = cpool.tile([128, 2, DIM], bf16)
            nc.scalar.dma_start(wo_sb[:], wo_in[:])
            bq_sb = cpool.tile([128, 2], f32)
            nc.scalar.dma_start(bq_sb[:], bq_in[:])
            bk_sb = cpool.tile([128, 1], f32)
            nc.scalar.dma_start(bk_sb[:], bk_in[:])
            bv_sb = cpool.tile([128, 1], f32)
            nc.scalar.dma_start(bv_sb[:], bv_in[:])
            cos_sb = cpool.tile([128, S], f32)
            nc.scalar.dma_start(cos_sb[:], cos_in[:])
            sin_sb = cpool.tile([128, S], f32)
            nc.scalar.dma_start(sin_sb[:], sin_in[:])
            mask_sb = cpool.tile([128, 4, 512], f32)
            nc.scalar.dma_start(mask_sb[:], mask_in.rearrange("p (r q) -> p r q", r=4))
            ones_f = cpool.tile([128, 128], f32)
            nc.vector.memset(ones_f[:], 1.0)
            ones_mat = cpool.tile([128, 128], bf16)
            nc.vector.tensor_copy(out=ones_mat[:], in_=ones_f[:])
            ident = cpool.tile([128, 128], f32)
            make_identity(nc, ident[:])

            # ---- persistent per-chunk activations (bf16 matmul operands)
            q_rot = [[apool.tile([128, 512], bf16, tag=f"qrot{h}_{c}", name=f"qrot{h}_{c}")
                      for c in range(4)] for h in range(2)]
            k_rot = [apool.tile([128, 512], bf16, tag=f"krot{c}", name=f"krot{c}")
                     for c in range(4)]
            v_sb = [apool.tile([128, 512], bf16, tag=f"vsb{c}", name=f"vsb{c}")
                    for c in range(4)]
            ctxT = [[apool.tile([128, 512], bf16, tag=f"ctx{h}_{c}", name=f"ctx{h}_{c}")
                     for c in range(4)] for h in range(2)]

            def rope(dst, raw, sc):
                # dst = raw*cos + swap(raw)*sinMod; sinMod has the -1 on the
                # low half baked in host-side (rotate_half sign).
                # math in f32, single rounding into the bf16 dst.
                ssl = slice(sc * 512, sc * 512 + 512)
                swp = wpool.tile([128, 512], f32, tag="ropeswp")
                nc.vector.tensor_copy(out=swp[0:64, :], in_=raw[64:128, :])
                nc.vector.tensor_copy(out=swp[64:128, :], in_=raw[0:64, :])
                tmp = wpool.tile([128, 512], f32, tag="ropetmp")
                nc.vector.tensor_tensor(tmp[:], swp[:], sin_sb[:, ssl], OP.mult)
                acc = wpool.tile([128, 512], f32, tag="ropeacc")
                nc.vector.tensor_tensor(acc[:], raw[:], cos_sb[:, ssl], OP.mult)
                nc.vector.tensor_tensor(acc[:], acc[:], tmp[:], OP.add)
                nc.vector.tensor_copy(out=dst[:], in_=acc[:])

            def proj_mm(sc, first=False, scores=None):
                # projection matmuls for sequence chunk sc; returns live
                # psums. scores=(qc, attnTs, pairs): interleave the previous
                # chunk's score/exp matmuls (SBUF-only, no DMA deps) after
                # each contract-pair so the PE never stalls on input tiles.
                ssl = slice(sc * 512, sc * 512 + 512)
                pq0 = ps1.tile([128, 512], f32, tag="A")
                pq1 = ps1.tile([128, 512], f32, tag="B")
                pk = ps1.tile([128, 512], f32, tag="C")
                pv = ps1.tile([128, 512], f32, tag="D")
                for cp in range(8):
                    if scores is not None:
                        qc_s, attnTs, pairs = scores
                        for h_s, kt_s in pairs[cp * len(pairs) // 8:(cp + 1) * len(pairs) // 8]:
                            emit_score(h_s, kt_s, qc_s, attnTs[h_s])
                    if first and cp < 7:
                        lo, hi = 2 * cp + 2, 2 * cp + 4
                        nc.sync.dma_start(wq_sb[:, lo:hi], wq_in[:, lo:hi])
                        nc.sync.dma_start(wk_sb[:, lo:hi], wk_in[:, lo:hi])
                        nc.sync.dma_start(wv_sb[:, lo:hi], wv_in[:, lo:hi])
                    qt = spool.tile([128, 2, 512], bf16, tag="qs")
                    nc.sync.dma_start(qt[:], qT_in[cp, sc])
                    kt_ = spool.tile([128, 2, 512], bf16, tag="ks")
                    nc.sync.dma_start(kt_[:], kT_in[cp, sc])
                    vt = spool.tile([128, 2, 512], bf16, tag="vs")
                    nc.sync.dma_start(vt[:], vT_in[cp, sc])
                    for t in range(2):
                        cc = 2 * cp + t
                        st, sp = cc == 0, cc == 15
                        nc.tensor.matmul(pq0[:], wq_sb[:, cc, 0:128],
                                         qt[:, t], start=st, stop=sp)
                        nc.tensor.matmul(pq1[:], wq_sb[:, cc, 128:256],
                                         qt[:, t], start=st, stop=sp)
                        nc.tensor.matmul(pk[:], wk_sb[:, cc],
                                         kt_[:, t], start=st, stop=sp)
                        nc.tensor.matmul(pv[:], wv_sb[:, cc],
                                         vt[:, t], start=st, stop=sp)
                return pq0, pq1, pk, pv

            def proj_epilogue(sc, pq0, pq1, pk, pv):
                # All four biases first (frees the proj psum banks early),
                # then the v transposes + their DVE copies at the HEAD of the
                # DVE queue, then RoPE. With ropes first, transpose j>=1
                # stalled the whole PE FIFO ~3us per phase waiting for its
                # psum slot behind ~10us of queued RoPE (the v3 phase gaps).
                q0_raw = wpool.tile([128, 512], f32, tag="rawq0")
                nc.scalar.activation(q0_raw[:], pq0[:], AF.Identity, bias=bq_sb[:, 0:1])
                q1_raw = wpool.tile([128, 512], f32, tag="rawq1")
                nc.scalar.activation(q1_raw[:], pq1[:], AF.Identity, bias=bq_sb[:, 1:2])
                k_raw = wpool.tile([128, 512], f32, tag="rawk")
                nc.scalar.activation(k_raw[:], pk[:], AF.Identity, bias=bk_sb[:])
                v_raw = wpool.tile([128, 512], f32, tag="rawv")
                nc.scalar.activation(v_raw[:], pv[:], AF.Identity, bias=bv_sb[:])
                for j in range(4):
                    ptr = ps1.tile([128, 128], f32, tag="A")
                    nc.tensor.transpose(ptr[:], v_raw[:, j * 128:(j + 1) * 128], ident[:])
                    nc.vector.tensor_copy(
                        out=v_sb[sc][:, j * 128:j * 128 + 128], in_=ptr[:])
                return q0_raw, q1_raw, k_raw

            def proj_ropes(sc, q0_raw, q1_raw, k_raw):
                # emitted LAST in each phase: the ~12us of RoPE DVE work sits
                # behind the attention recip/copies in the DVE FIFO, and
                # finishes during the next proj chunk's DMA-paced matmuls.
                rope(q_rot[0][sc], q0_raw, sc)
                rope(q_rot[1][sc], q1_raw, sc)
                rope(k_rot[sc], k_raw, sc)

            def emit_score(h, kt, qc, attnT):
                pst = ps2.tile([128, 512], f32, tag="sT")
                nc.tensor.matmul(
                    pst[:], k_rot[kt // 4][:, (kt % 4) * 128:(kt % 4) * 128 + 128],
                    q_rot[h][qc][:], start=True, stop=True)
                r = kt - 4 * qc
                if r >= 0:
                    nc.vector.tensor_tensor(pst[:], pst[:], mask_sb[:, r], OP.add)
                nc.scalar.activation(attnT[:, kt], pst[:], AF.Exp, scale=float(SCALE))

            def attn(qc, pre=None):
                # attention for query chunk qc, both heads. If pre is given,
                # the scores/exps were already emitted interleaved into the
                # projection stream; only the sums/normalize remain.
                n_kt = 4 * (qc + 1)
                if pre is None:
                    # hoist BOTH heads' scores+exps ahead of the sums: h1's
                    # exps stream on ACT while h0's sums occupy the PE, so
                    # the final block's sums never chase the exp frontier
                    # (v15's tail showed ~8us of S[157]/S[166] exp-wait
                    # stalls at t=193-211us). Same two-slot attnT pattern
                    # the interleaved phases already use.
                    pre = [atpool.tile([128, 16, 512], bf16, tag="attnT",
                                       name=f"atF{h}_{qc}") for h in range(2)]
                    for h in range(2):
                        for kt in range(n_kt):
                            emit_score(h, kt, qc, pre[h])
                for h in range(2):
                    attnT = pre[h]
                    psum = ps1.tile([128, 512], f32, tag="C" if h == 0 else "A")
                    pctx = ps1.tile([128, 512], f32, tag="B" if h == 0 else "D")
                    for kt in range(n_kt):
                        nc.tensor.matmul(psum[:], ones_mat[:],
                                         attnT[:, kt],
                                         start=kt == 0, stop=kt == n_kt - 1)
                        nc.tensor.matmul(pctx[:], v_sb[kt // 4][:, (kt % 4) * 128:(kt % 4) * 128 + 128],
                                         attnT[:, kt],
                                         start=kt == 0, stop=kt == n_kt - 1)
                    # normalize in 128-column groups: out-proj tile st only
                    # needs ctxT columns (st%4)*128..+128, so releasing each
                    # group early unhides the 3.4us full-width reciprocal
                    # from the phase-end critical path.
                    for g4 in range(4):
                        csl = slice(g4 * 128, g4 * 128 + 128)
                        bc_sb = wpool.tile([128, 128], f32, tag="bc")
                        nc.vector.reciprocal(out=bc_sb[:], in_=psum[:, csl])
                        nc.vector.tensor_tensor(
                            ctxT[h][qc][:, csl], pctx[:, csl], bc_sb[:], OP.mult)

            def outproj(qc):
                # out-proj rows for the 4 seq tiles of query chunk qc
                for st in range(4 * qc, 4 * qc + 4):
                    tsl = slice((st % 4) * 128, (st % 4) * 128 + 128)
                    ot = wpool.tile([128, 2048], bf16, tag="ot")
                    for ec in range(4):
                        esl = slice(ec * 512, ec * 512 + 512)
                        po = ps2.tile([128, 512], f32, tag="po")
                        nc.tensor.matmul(po[:], ctxT[0][qc][:, tsl],
                                         wo_sb[:, 0, esl], start=True, stop=False)
                        nc.tensor.matmul(po[:], ctxT[1][qc][:, tsl],
                                         wo_sb[:, 1, esl], start=False, stop=True)
                        if ec % 2 == 0:
                            nc.vector.tensor_copy(out=ot[:, esl], in_=po[:])
                        else:
                            nc.scalar.activation(ot[:, esl], po[:], AF.Copy)
                    nc.sync.dma_start(
                        out_dram[st * 128:st * 128 + 128, :], ot[:])

            # ---- HAM-aware interleave: keep the PE stream dense.
            # proj(0), epi(0), proj(1), epi(1)+attn(0)+out(0), proj(2), ...
            ps = proj_mm(0, first=True)
            raws = proj_epilogue(0, *ps)
            proj_ropes(0, *raws)
            for sc in range(1, 4):
                qc = sc - 1
                at0 = atpool.tile([128, 16, 512], bf16, tag="attnT", name=f"at0_{sc}")
                at1 = atpool.tile([128, 16, 512], bf16, tag="attnT", name=f"at1_{sc}")
                pairs = [(h, kt) for h in range(2) for kt in range(4 * (qc + 1))]
                ps = proj_mm(sc, scores=(qc, [at0, at1], pairs))
                raws = proj_epilogue(sc, *ps)
                attn(qc, pre=[at0, at1])
                outproj(qc)
                proj_ropes(sc, *raws)
            attn(3)
            outproj(3)
    _split_multi_waits(nc)
    return nc


def kernel(query, key, value, Wq, bq, Wk, bk, Wv, bv, Wo, bo):
    from concourse.bass_utils import run_bass_kernel_spmd

    query = np.asarray(query, np.float32)
    key = np.asarray(key, np.float32)
    value = np.asarray(value, np.float32)
    B = query.shape[0]

    def _tile_in(x):
        # [S, DIM] -> [cp, sc, ci, two, s]: per-(cp, sc) DMA block is one
        # contiguous [128, 1024] run (2KB per partition line).
        a = _bf16(x.reshape(S, DIM).T).reshape(8, 2, 128, 4, 512)
        return np.ascontiguousarray(a.transpose(0, 3, 2, 1, 4))

    qT = _tile_in(query)
    kT = _tile_in(key)
    vT = _tile_in(value)
    cosT, sinT = _rope_cos_sin_T()
    sinT = sinT.copy()
    sinT[0:64, :] *= -1.0  # rotate_half: low half gets -x2*sin
    sinT = np.ascontiguousarray(sinT)
    masks = _masks()

    if "nc" not in _F32R_CACHE:
        _F32R_CACHE["nc"] = _build_program()
    nc = _F32R_CACHE["nc"]

    in_maps = []
    for i in range(N_CORES):
        g = i // 2
        # weight slices pretiled to [ci, co, d] / [d, h, e]: contiguous
        # multi-KB per-partition runs for the preload DMAs.
        Wq_s = np.ascontiguousarray(
            _bf16(np.asarray(Wq, np.float32)[256 * i:256 * (i + 1), :].T)
            .reshape(16, 128, 256).transpose(1, 0, 2))
        Wk_s = np.ascontiguousarray(
            _bf16(np.asarray(Wk, np.float32)[128 * g:128 * (g + 1), :].T)
            .reshape(16, 128, 128).transpose(1, 0, 2))
        Wv_s = np.ascontiguousarray(
            _bf16(np.asarray(Wv, np.float32)[128 * g:128 * (g + 1), :].T)
            .reshape(16, 128, 128).transpose(1, 0, 2))
        Wo_s = np.ascontiguousarray(
            _bf16(np.asarray(Wo, np.float32)[:, 256 * i:256 * (i + 1)].T)
            .reshape(2, 128, DIM).transpose(1, 0, 2))
        bq_c = np.ascontiguousarray(np.asarray(bq, np.float32)[256 * i:256 * (i + 1)].reshape(2, 128).T)
        bk_c = np.asarray(bk, np.float32)[128 * g:128 * (g + 1)].reshape(128, 1)
        bv_c = np.asarray(bv, np.float32)[128 * g:128 * (g + 1)].reshape(128, 1)
        in_maps.append({
            "queryT": qT, "keyT": kT, "valueT": vT,
            "wqT": Wq_s, "wkT": Wk_s, "wvT": Wv_s, "woT": Wo_s,
            "bq_col": bq_c, "bk_col": np.ascontiguousarray(bk_c),
            "bv_col": np.ascontiguousarray(bv_c),
            "cosT": cosT, "sinT": sinT, "masks": masks,
        })

    _F32R_CACHE["in_maps"] = in_maps
    globals()["_LAST_IN_MAPS"] = in_maps
    res = run_bass_kernel_spmd(nc, in_maps, list(range(N_CORES)))
    out = res.results[0]["partial"].astype(np.float32)
    for i in range(1, N_CORES):
        out = out + res.results[i]["partial"].astype(np.float32)
    out = out + np.asarray(bo, np.float32)[None, :]
    return out.reshape(B, S, DIM).astype(np.float32)
